# revision 27
# baseline (speedup 1.0000x reference)
"""Trainium2 Bass kernel for Luong-style attention.

Reference computation (per full problem):
    h = decoder_hidden @ W.T + b          # [B, De]
    enc = encoder_output.transpose(1,0,2) # [B, S, De]
    a = softmax(einsum('bsd,bd->bs', enc, h), axis=1)
    context = einsum('bs,bsd->bd', a, enc)  # [B, De]

Shapes: B=64, S=4096, Dd=1024, De=512 (f32).

Strategy: data-parallel over B across 8 NeuronCores (B_local=8 each).
encoder_output is the huge tensor (512 MB); each core streams its
64 MB shard from HBM exactly once (chunked two-level softmax).
Per 128-row s-tile (f32, no bf16 copy of the stream):
  - scores via DVE scalar_tensor_tensor (product + row-sum in one op)
    against a partition-broadcast fp16 copy of h (one 2-byte source
    keeps the DVE at full rate; two f32 sources would halve it),
  - per-chunk softmax via PE transpose + ACT exp (bias=-chunk_max,
    fused row-sum),
  - context accumulated in a single [8, 512] PSUM bank: for each b,
    matmul(lhsT=prob column [128,1] fp32r, rhs=enc f32r [128,512]) ->
    out row [1,512] at partition b.  fp32r moving data runs at
    1 cycle/row for N>=256, and a 1-column weight load is ~free, so
    the PE streams the f32 tile directly (no bf16 cast needed).
    The accumulation group is opened by one bank-wide zero matmul
    (start=True clears has_written bits bank-wide).
  - chunks combined at the end with exp(m_c - M)/l weights; partials
    are already in natural [b, d] layout so the combine is a short
    chain of [8,512] DVE ops.
No collectives needed.  Bottleneck is the HBM stream (~187 us for
64 MB/core at ~358 GB/s); all engines sit below that.
"""

import numpy as np

import concourse.bass as bass
import concourse.bacc as bacc_mod
import concourse.tile as tile
import concourse.mybir as mybir
from concourse import masks
from concourse.bass_utils import run_bass_kernel_spmd

F32 = mybir.dt.float32
F32R = mybir.dt.float32r
F16 = mybir.dt.float16
BF16 = mybir.dt.bfloat16
ALU = mybir.AluOpType
ACTF = mybir.ActivationFunctionType
AX = mybir.AxisListType

NCORES = 8
B = 8          # per-core batch
S = 4096
DD = 1024
DE = 512
P = 128        # s-values per tile
NTILES = S // P          # 32
CHUNK_TILES = 4          # s-tiles per softmax chunk
NCHUNK = NTILES // CHUNK_TILES   # 8


def build_nc(ntiles: int = NTILES):
    nchunk = ntiles // CHUNK_TILES
    s_local = ntiles * P
    nc = bacc_mod.Bacc("TRN2", target_bir_lowering=False, debug=False)
    dec_d = nc.dram_tensor("decoder_hidden", [B, DD], F32, kind="ExternalInput")
    enc_d = nc.dram_tensor("encoder_output", [s_local, B, DE], F32, kind="ExternalInput")
    w_d = nc.dram_tensor("W", [DE, DD], F32, kind="ExternalInput")
    b_d = nc.dram_tensor("b", [DE], F32, kind="ExternalInput")
    out_d = nc.dram_tensor("out", [B, DE], F32, kind="ExternalOutput")

    with tile.TileContext(nc) as tc:
        with (
            tc.tile_pool(name="const", bufs=1) as const_pool,
            tc.tile_pool(name="persist", bufs=1) as persist_pool,
            tc.tile_pool(name="enc", bufs=4) as enc_pool,
            tc.tile_pool(name="ebf", bufs=6) as ebf_pool,
            tc.tile_pool(name="junk", bufs=4) as junk_pool,
            tc.tile_pool(name="scores", bufs=6) as sc_pool,
            tc.tile_pool(name="probs", bufs=2) as p_pool,
            tc.tile_pool(name="pt", bufs=4) as pt_pool,
            tc.tile_pool(name="acctmp", bufs=2) as acc_tmp_pool,
        ):
            wload_cm = tc.tile_pool(name="wload", bufs=2)
            wload_pool = wload_cm.__enter__()
            wt_cm = tc.tile_pool(name="wt", bufs=1)
            wt_pool = wt_cm.__enter__()
            setup_psum_cm = tc.tile_pool(name="psum_setup", bufs=4, space="PSUM")
            psum_setup = setup_psum_cm.__enter__()
            setup_psum2_cm = tc.tile_pool(name="psum_setup2", bufs=4, space="PSUM")
            psum_setup2 = setup_psum2_cm.__enter__()
            # ---- constants ----
            ident = const_pool.tile([P, P], F32)
            masks.make_identity(nc, ident[:])
            ones = const_pool.tile([1, P], F32)
            nc.vector.memset(ones[:], 1.0)
            zeros_row = const_pool.tile([1, DE], BF16)
            nc.vector.memset(zeros_row[:], 0.0)
            ones_bf = const_pool.tile([1, B], BF16)
            nc.vector.memset(ones_bf[:], 1.0)
            # row-broadcast selectors: sel[:, bb, :] is [8, 128] with row bb
            # all-ones; matmul(sel_bb, x) broadcasts x's row bb to all
            # 128 partitions without any cross-partition DMA.
            sel = const_pool.tile([B, B, P], F32)
            nc.gpsimd.memset(sel[:], 0.0)
            # sel[k, bb, m] = 1.0 iff k == bb  (k*1 + bb*(-1) == 0)
            nc.gpsimd.affine_select(
                out=sel[:], in_=sel[:],
                compare_op=ALU.not_equal, fill=1.0, base=0,
                pattern=[[-1, B], [0, P]], channel_multiplier=1)

            # ---- load small inputs ----
            dec_sb = const_pool.tile([B, DD], F32)
            nc.sync.dma_start(dec_sb[:], dec_d[:])
            bias_sb = const_pool.tile([1, DE], F32)
            nc.sync.dma_start(bias_sb[:], b_d[None, :])
            setup_dmas = []

            # ---- transpose dec: [8,1024] -> decT [128, 8, 8] (chunk c = cols c*128..) ----
            # f32r so the h matmuls stream at 1 cycle/row instead of 4
            decT = const_pool.tile([P, DD // P, B], F32R)
            for c in range(DD // P):
                tp = psum_setup.tile([P, B], F32, tag="su")
                nc.tensor.transpose(tp[:], dec_sb[:, c * P:(c + 1) * P], ident[0:B, 0:B])
                nc.scalar.copy(decT[:, c, :], tp[:])

            # ---- transpose W: [512,1024] -> WT [128, 8, 512] (chunk c = W.T rows c*128..) ----
            # 4 transposes drain into one [128, 512] psum bank -> 1 big copy,
            # alternating Vector/Scalar so neither queue serializes the head.
            wt_sb = wt_pool.tile([P, DD // P, DE], F32R)
            for wi in range(DE // P):
                w_row = wload_pool.tile([P, DD], F32, tag="wrow")
                half = DD // 2
                setup_dmas.append(nc.sync.dma_start(
                    w_row[:, 0:half], w_d[wi * P:(wi + 1) * P, 0:half]))
                setup_dmas.append(nc.sync.dma_start(
                    w_row[:, half:DD], w_d[wi * P:(wi + 1) * P, half:DD]))
                for g in range(2):
                    tp = psum_setup.tile([P, 4 * P], F32, tag="su")
                    for c in range(4):
                        nc.tensor.transpose(tp[:, c * P:(c + 1) * P],
                                            w_row[:, (4 * g + c) * P:(4 * g + c + 1) * P],
                                            ident[:])
                    eng = nc.vector.tensor_copy if (wi * 2 + g) % 2 else nc.scalar.copy
                    eng(wt_sb[:, 4 * g:4 * g + 4, wi * P:(wi + 1) * P],
                        tp[:].rearrange("p (c d) -> p c d", c=4))

            # ---- h = dec @ W.T + b  -> h_sb [8, 512] ----
            h_ps = psum_setup2.tile([B, DE], F32, tag="hsu")
            for c in range(DD // P):
                nc.tensor.matmul(h_ps[:], decT[:, c, :], wt_sb[:, c, :],
                                 start=(c == 0), stop=False)
            nc.tensor.matmul(h_ps[:], ones[0:1, 0:B], bias_sb[:],
                             start=False, stop=True)
            h_sb = const_pool.tile([B, DE], F32)
            nc.scalar.copy(h_sb[:], h_ps[:])

            # ---- broadcast h along partitions: hb [128, 8, 512] fp16 ----
            # selector matmul: out = sel_bb.T @ h_sb puts h row bb on all
            # 128 partitions; the psum->sbuf copy converts to fp16 so the
            # score stt has only one 4-byte source (full DVE rate).
            hb = persist_pool.tile([P, B, DE], BF16)
            for bb in range(B):
                hp = psum_setup2.tile([P, DE], F32, tag="hsu")
                nc.tensor.matmul(hp[:], sel[:, bb, :], h_sb[:],
                                 start=True, stop=True)
                nc.scalar.copy(hb[:, bb, :], hp[:])

            setup_psum2_cm.__exit__(None, None, None)
            setup_psum_cm.__exit__(None, None, None)
            wt_cm.__exit__(None, None, None)
            wload_cm.__exit__(None, None, None)
            _tr_cm = tc.tile_pool(name="psum_tr", bufs=2, space="PSUM")
            psum_tr = _tr_cm.__enter__()
            _sc_cm = tc.tile_pool(name="psum_sc", bufs=2, space="PSUM")
            psum_sc = _sc_cm.__enter__()
            _ctx_cm = tc.tile_pool(name="psum_ctx", bufs=3, space="PSUM")
            psum_ctx = _ctx_cm.__enter__()

            # ---- diagonal prob-weight tiles ----
            # pz[p, b, b'] = prob_col_b[p] iff b' == b else 0.  pz[:, b, :]
            # is a [128, 8] bf16 weight whose single nonzero column routes
            # batch b's context row to psum partition b while keeping the
            # matmul base partition at 0.  Off-diagonals are zeroed once here
            # and never written again; each tile only refreshes the 8
            # diagonal slots (stride-9 AP).
            NPZ = 4
            pzs = [persist_pool.tile([P, B, B], BF16, name=f"pz{i}")
                   for i in range(NPZ)]
            pz_diags = []
            for pz in pzs:
                nc.vector.memset(pz[:], 0.0)
                pz_diags.append(pz[:].rearrange("p a b -> p (a b)")[:, 0:B * B:B + 1])

            # ---- online-softmax running state (flash-attention style) ----
            # Chunks of 4 tiles, then 1-tile mini-chunks at the end so the
            # post-stream tail is one small softmax + 8 matmuls, not a whole
            # chunk + combine chain.
            chunk_sizes = [CHUNK_TILES] * (ntiles // CHUNK_TILES - 1) + [2, 1, 1]
            Mr = [persist_pool.tile([B, 1], F32, name=f"Mrun{i}") for i in range(4)]
            lr = [persist_pool.tile([B, 1], F32, name=f"lrun{i}") for i in range(2)]
            acc = [persist_pool.tile([B, DE], F32, name=f"acc{i}") for i in range(2)]

            # ---- main streaming loop over S ----
            # The fold of chunk c into the running accumulator depends on
            # chunk c's matmuls; emitting it right away would park the
            # in-order DVE queue on that dependency and stall the next
            # chunk's score ops.  Instead each chunk's fold is DEFERRED
            # until after the next chunk's scores are emitted, by which
            # time the matmuls have long finished.
            pending_folds = []
            j = 0
            for c, ct in enumerate(chunk_sizes):
                # open the ctx accumulation bank early (no deps beyond pool
                # rotation) so the PE's in-order queue never stalls on it.
                ctx_ps = psum_ctx.tile([B, DE], F32)
                nc.tensor.matmul(ctx_ps[:], ones_bf[:], zeros_row[:],
                                 start=True, stop=False)
                prod_tiles = []
                scT = psum_sc.tile([B, CHUNK_TILES * P], F32)
                for t in range(ct):
                    et = enc_pool.tile([P, B, DE], F32)
                    enc_dma = nc.sync.dma_start(et[:], enc_d[(j + t) * P:(j + t + 1) * P, :, :])
                    if j + t == 0:
                        for sd in setup_dmas:
                            tile.add_dep_helper(enc_dma.ins, sd.ins,
                                                reason="let setup W loads win HBM first")
                    # bf16 copy of the tile on the (otherwise idle) Scalar
                    # engine: the DVE score op then runs with two bf16
                    # sources (full/2x rate), and the same copy is the
                    # context matmul's moving operand.
                    etb = ebf_pool.tile([P, B, DE], BF16, tag="ebf")
                    nc.scalar.copy(etb[:], et[:])
                    prod_tiles.append(etb)
                    sct = sc_pool.tile([P, B], F32)
                    for bb in range(B):
                        junk = junk_pool.tile([P, DE], BF16, tag="junk")
                        nc.vector.scalar_tensor_tensor(
                            out=junk[:],
                            in0=etb[:, bb, :],
                            scalar=1.0,
                            in1=hb[:, bb, :],
                            op0=ALU.mult,
                            op1=ALU.mult,
                            accum_out=sct[:, bb:bb + 1],
                        )
                    # transpose scores into [8, 128] slice of chunk psum
                    nc.tensor.transpose(scT[:, t * P:(t + 1) * P], sct[:], ident[:])

                # fold from two chunks back: its matmuls finished long ago,
                # so the fold's psum read never blocks an engine queue.
                if len(pending_folds) == 2:
                    pending_folds.pop(0)()

                # chunk softmax against the RUNNING max (flash style): the
                # chunk's probs are already scaled by exp(m - M_new), so the
                # psum partial needs no per-chunk reweighting later; only the
                # accumulator gets rescaled by alpha = exp(M_old - M_new).
                m_c = sc_pool.tile([B, 1], F32, tag="stat")
                nc.vector.reduce_max(m_c[:], scT[:, 0:ct * P], axis=AX.X)
                negm = sc_pool.tile([B, 1], F32, tag="stat")
                l_c = sc_pool.tile([B, 1], F32, tag="stat")
                if c == 0:
                    nc.vector.tensor_copy(Mr[0][:], m_c[:])
                else:
                    nc.vector.tensor_tensor(out=Mr[c % 4][:], in0=Mr[(c - 1) % 4][:],
                                            in1=m_c[:], op=ALU.max)
                nc.vector.tensor_scalar_mul(negm[:], Mr[c % 4][:], -1.0)
                p_sb = p_pool.tile([B, CHUNK_TILES * P], F32)
                nc.scalar.activation(p_sb[:, 0:ct * P], scT[:, 0:ct * P], ACTF.Exp,
                                     bias=negm[:], scale=1.0,
                                     accum_out=l_c[:])

                # context partial: ctx[b, :] += sum_s p[s, b] * prod[s, b, :]
                # accumulated in the one [8, 512] psum bank opened above
                for t in range(ct):
                    ptp = psum_tr.tile([P, B], F32, tag="tr")
                    nc.tensor.transpose(ptp[:], p_sb[:, t * P:(t + 1) * P], ident[0:B, 0:B])
                    pz, pzd = pzs[(j + t) % NPZ], pz_diags[(j + t) % NPZ]
                    nc.scalar.copy(pzd, ptp[:])
                    for bb in range(B):
                        nc.tensor.matmul(
                            ctx_ps[:],
                            pz[:, bb, :],
                            prod_tiles[t][:, bb, :],
                            start=False,
                            stop=(t == ct - 1 and bb == B - 1))

                def make_fold(c=c, ctx_ps=ctx_ps, l_c=l_c, negm=negm):
                    def fold():
                        if c == 0:
                            nc.vector.tensor_copy(acc[0][:], ctx_ps[:])
                            nc.vector.tensor_copy(lr[0][:], l_c[:])
                        else:
                            alpha = sc_pool.tile([B, 1], F32, tag="stat")
                            nc.scalar.activation(alpha[:], Mr[(c - 1) % 4][:], ACTF.Exp,
                                                 bias=negm[:], scale=1.0)
                            nc.vector.scalar_tensor_tensor(
                                out=lr[c % 2][:], in0=lr[(c - 1) % 2][:],
                                scalar=alpha[:, 0:1],
                                in1=l_c[:], op0=ALU.mult, op1=ALU.add)
                            nc.vector.scalar_tensor_tensor(
                                out=acc[c % 2][:], in0=acc[(c - 1) % 2][:],
                                scalar=alpha[:, 0:1],
                                in1=ctx_ps[:], op0=ALU.mult, op1=ALU.add)
                    return fold
                pending_folds.append(make_fold())
                j += ct
            for f in pending_folds:
                f()

            # ---- finalize: divide by l_total and by h_q, store ----
            nchunks_total = len(chunk_sizes)
            last = (nchunks_total - 1) % 2
            g_rl = persist_pool.tile([B, 1], F32)
            nc.vector.reciprocal(g_rl[:], lr[last][:])
            final_sb = persist_pool.tile([B, DE], F32)
            nc.vector.tensor_scalar(out=final_sb[:], in0=acc[last][:],
                                    scalar1=g_rl[:, 0:1], scalar2=None,
                                    op0=ALU.mult)
            nc.sync.dma_start(out_d[:], final_sb[:])
            _ctx_cm.__exit__(None, None, None)
            _sc_cm.__exit__(None, None, None)
            _tr_cm.__exit__(None, None, None)

    nc.compile()
    if not nc.is_finalized():
        nc.finalize()
    return nc


_NC = None


def kernel(decoder_hidden, encoder_output, W, b):
    global _NC
    if _NC is None:
        _NC = build_nc()
    decoder_hidden = np.ascontiguousarray(decoder_hidden, dtype=np.float32)
    encoder_output = np.ascontiguousarray(encoder_output, dtype=np.float32)
    W = np.ascontiguousarray(W, dtype=np.float32)
    b = np.ascontiguousarray(b, dtype=np.float32)

    in_maps = []
    for i in range(NCORES):
        sl = slice(i * B, (i + 1) * B)
        in_maps.append({
            "decoder_hidden": decoder_hidden[sl],
            "encoder_output": np.ascontiguousarray(encoder_output[:, sl, :]),
            "W": W,
            "b": b,
        })
    res = run_bass_kernel_spmd(_NC, in_maps, core_ids=list(range(NCORES)))
    return np.concatenate([res.results[i]["out"] for i in range(NCORES)], axis=0)


# revision 28
# speedup vs baseline: 1.2465x; 1.2465x over previous
"""Trainium2 Bass kernel for Luong-style attention.

Reference computation (per full problem):
    h = decoder_hidden @ W.T + b          # [B, De]
    enc = encoder_output.transpose(1,0,2) # [B, S, De]
    a = softmax(einsum('bsd,bd->bs', enc, h), axis=1)
    context = einsum('bs,bsd->bd', a, enc)  # [B, De]

Shapes: B=64, S=4096, Dd=1024, De=512 (f32).

Strategy: data-parallel over B across 8 NeuronCores (B_local=8 each).
encoder_output is the huge tensor (512 MB); each core streams its
64 MB shard from HBM exactly once (chunked two-level softmax):
  - scores in f32 via fused DVE scalar_tensor_tensor (product +
    row-sum in one op) against a partition-broadcast copy of h,
  - each tile also cast f32->bf16 on the Scalar engine for the
    context path (bf16 weights make PE LDWEIGHTS ~5x faster),
  - per-chunk softmax via PE transpose + ACT exp (bias=-chunk_max,
    fused row-sum),
  - context accumulated TRANSPOSED in a single PSUM bank
    (lhsT = bf16 enc d-slice as weights, rhs = prob column, N=1;
    the accumulation group is opened by one bank-wide zero matmul
    because start=True clears has_written bits bank-wide),
  - chunks combined with exp(m_c - M)/l weights at the end,
    partition-broadcasts done with one-hot selector matmuls (no
    cross-partition DMA hops), then transposed back and stored.
No collectives needed.  ~228 us on silicon vs ~179 us single-pass
HBM roofline (64 MB/core at ~358 GB/s); rel err 1.7e-3.
"""

import numpy as np

import concourse.bass as bass
import concourse.bacc as bacc_mod
import concourse.tile as tile
import concourse.mybir as mybir
from concourse import masks
from concourse.bass_utils import run_bass_kernel_spmd

F32 = mybir.dt.float32
BF16 = mybir.dt.bfloat16
ALU = mybir.AluOpType
ACTF = mybir.ActivationFunctionType
AX = mybir.AxisListType

NCORES = 8
B = 8          # per-core batch
S = 4096
DD = 1024
DE = 512
P = 128        # s-values per tile
M = DE // P              # 4 d-chunks
NTILES = S // P          # 32
CHUNK_TILES = 4          # s-tiles per softmax chunk
NCHUNK = NTILES // CHUNK_TILES   # 8


def build_nc(ntiles: int = NTILES):
    nchunk = ntiles // CHUNK_TILES
    s_local = ntiles * P
    nc = bacc_mod.Bacc("TRN2", target_bir_lowering=False, debug=False)
    dec_d = nc.dram_tensor("decoder_hidden", [B, DD], F32, kind="ExternalInput")
    enc_d = nc.dram_tensor("encoder_output", [s_local, B, DE], F32, kind="ExternalInput")
    w_d = nc.dram_tensor("W", [DE, DD], F32, kind="ExternalInput")
    b_d = nc.dram_tensor("b", [DE], F32, kind="ExternalInput")
    out_d = nc.dram_tensor("out", [B, DE], F32, kind="ExternalOutput")

    with tile.TileContext(nc) as tc:
        with (
            tc.tile_pool(name="const", bufs=1) as const_pool,
            tc.tile_pool(name="wload", bufs=4) as wload_pool,
            tc.tile_pool(name="persist", bufs=1) as persist_pool,
            tc.tile_pool(name="enc", bufs=5) as enc_pool,
            tc.tile_pool(name="encbf", bufs=5) as encbf_pool,
            tc.tile_pool(name="scratch", bufs=4) as scratch_pool,
            tc.tile_pool(name="scores", bufs=4) as sc_pool,
            tc.tile_pool(name="probs", bufs=2) as p_pool,
            tc.tile_pool(name="pt", bufs=6) as pt_pool,
        ):
            setup_psum_cm = tc.tile_pool(name="psum_setup", bufs=4, space="PSUM")
            psum_setup = setup_psum_cm.__enter__()
            setup_psum2_cm = tc.tile_pool(name="psum_setup2", bufs=4, space="PSUM")
            psum_setup2 = setup_psum2_cm.__enter__()
            # ---- constants ----
            ident = const_pool.tile([P, P], F32)
            masks.make_identity(nc, ident[:])
            ones = const_pool.tile([1, P], F32)
            nc.vector.memset(ones[:], 1.0)
            zeros_row = const_pool.tile([1, M * B], BF16)
            nc.vector.memset(zeros_row[:], 0.0)
            ones_bf = const_pool.tile([1, P], BF16)
            nc.vector.memset(ones_bf[:], 1.0)
            # row-broadcast selectors: sel[:, bb, :] is [8, 128] with row bb
            # all-ones; matmul(sel_bb, x) broadcasts x's row bb to all
            # 128 partitions without any cross-partition DMA.
            sel = const_pool.tile([B, B, P], F32)
            nc.gpsimd.memset(sel[:], 0.0)
            # sel[k, bb, m] = 1.0 iff k == bb  (k*1 + bb*(-1) == 0)
            nc.gpsimd.affine_select(
                out=sel[:], in_=sel[:],
                compare_op=ALU.not_equal, fill=1.0, base=0,
                pattern=[[-1, B], [0, P]], channel_multiplier=1)

            # ---- load small inputs ----
            dec_sb = const_pool.tile([B, DD], F32)
            nc.sync.dma_start(dec_sb[:], dec_d[:])
            bias_sb = const_pool.tile([1, DE], F32)
            nc.sync.dma_start(bias_sb[:], b_d[None, :])
            setup_dmas = []

            # ---- transpose dec: [8,1024] -> decT [128, 8, 8] (chunk c = cols c*128..) ----
            decT = const_pool.tile([P, DD // P, B], F32)
            for c in range(DD // P):
                tp = psum_setup.tile([P, B], F32, tag="su")
                nc.tensor.transpose(tp[:], dec_sb[:, c * P:(c + 1) * P], ident[0:B, 0:B])
                nc.vector.tensor_copy(decT[:, c, :], tp[:])

            # ---- transpose W: [512,1024] -> WT [128, 8, 512] (chunk c = W.T rows c*128..) ----
            wt_sb = persist_pool.tile([P, DD // P, DE], F32)
            for wi in range(DE // P):
                w_row = wload_pool.tile([P, DD], F32, tag="wrow")
                half = DD // 2
                setup_dmas.append(nc.sync.dma_start(
                    w_row[:, 0:half], w_d[wi * P:(wi + 1) * P, 0:half]))
                setup_dmas.append(nc.sync.dma_start(
                    w_row[:, half:DD], w_d[wi * P:(wi + 1) * P, half:DD]))
                for c in range(DD // P):
                    tp = psum_setup.tile([P, P], F32, tag="su")
                    nc.tensor.transpose(tp[:], w_row[:, c * P:(c + 1) * P], ident[:])
                    nc.vector.tensor_copy(wt_sb[:, c, wi * P:(wi + 1) * P], tp[:])

            # ---- h = dec @ W.T + b  -> h_sb [8, 512] ----
            h_ps = psum_setup2.tile([B, DE], F32, tag="hsu")
            for c in range(DD // P):
                nc.tensor.matmul(h_ps[:], decT[:, c, :], wt_sb[:, c, :],
                                 start=(c == 0), stop=False)
            nc.tensor.matmul(h_ps[:], ones[0:1, 0:B], bias_sb[:],
                             start=False, stop=True)
            h_sb = const_pool.tile([B, DE], F32)
            nc.vector.tensor_copy(h_sb[:], h_ps[:])

            # ---- broadcast h along partitions: hb [128, 8, 512] ----
            # selector matmul: out = sel_bb.T @ h_sb puts h row bb on all
            # 128 partitions; no cross-partition DMA hop in the chain.
            hb = persist_pool.tile([P, B, DE], F32)
            for bb in range(B):
                hp = psum_setup2.tile([P, DE], F32, tag="hsu")
                nc.tensor.matmul(hp[:], sel[:, bb, :], h_sb[:],
                                 start=True, stop=True)
                nc.vector.tensor_copy(hb[:, bb, :], hp[:])

            setup_psum2_cm.__exit__(None, None, None)
            setup_psum_cm.__exit__(None, None, None)
            _tr_cm = tc.tile_pool(name="psum_tr", bufs=4, space="PSUM")
            psum_tr = _tr_cm.__enter__()
            _sc_cm = tc.tile_pool(name="psum_sc", bufs=2, space="PSUM")
            psum_sc = _sc_cm.__enter__()
            _ctx_cm = tc.tile_pool(name="psum_ctx", bufs=2, space="PSUM")
            psum_ctx = _ctx_cm.__enter__()

            # ---- per-chunk stats / outputs ----
            m_all = persist_pool.tile([B, nchunk], F32)
            negm_all = persist_pool.tile([B, nchunk], F32)
            l_all = persist_pool.tile([B, nchunk], F32)
            w_all = persist_pool.tile([B, nchunk], F32)
            # transposed context partials: [de%128, chunk, m, b]
            ctxt_all = persist_pool.tile([P, nchunk, M, B], F32)

            # ---- main streaming loop over S ----
            for c in range(nchunk):
                enc_tiles = []
                scT = psum_sc.tile([B, CHUNK_TILES * P], F32)
                for t in range(CHUNK_TILES):
                    j = c * CHUNK_TILES + t
                    et = enc_pool.tile([P, B, DE], F32)
                    enc_dma = nc.sync.dma_start(et[:], enc_d[j * P:(j + 1) * P, :, :])
                    if j == 0:
                        for sd in setup_dmas:
                            tile.add_dep_helper(enc_dma.ins, sd.ins,
                                                reason="let setup W loads win HBM first")
                    et_bf = encbf_pool.tile([P, B, DE], BF16)
                    nc.scalar.copy(et_bf[:], et[:])
                    enc_tiles.append(et_bf)
                    # scores for this tile: [128, 8]
                    sct = sc_pool.tile([P, B], F32)
                    for bb in range(B):
                        junk = scratch_pool.tile([P, DE], BF16, tag="junk")
                        nc.vector.scalar_tensor_tensor(
                            out=junk[:],
                            in0=et[:, bb, :],
                            scalar=1.0,
                            in1=hb[:, bb, :],
                            op0=ALU.mult,
                            op1=ALU.mult,
                            accum_out=sct[:, bb:bb + 1],
                        )
                    # transpose scores into [8, 128] slice of chunk psum
                    nc.tensor.transpose(scT[:, t * P:(t + 1) * P], sct[:], ident[:])

                # chunk softmax: m_c, p_c, l_c
                nc.vector.reduce_max(m_all[:, c:c + 1], scT[:], axis=AX.X)
                nc.vector.tensor_scalar_mul(negm_all[:, c:c + 1], m_all[:, c:c + 1], -1.0)
                p_sb = p_pool.tile([B, CHUNK_TILES * P], F32)
                nc.scalar.activation(p_sb[:], scT[:], ACTF.Exp,
                                     bias=negm_all[:, c:c + 1], scale=1.0,
                                     accum_out=l_all[:, c:c + 1])

                # transposed context partial, all in ONE psum bank:
                # ctxT[p, m, b] += sum_s enc[s, b, m*128+p] * p_c[s, b]
                # `start=True` clears has_written bits bank-wide, so open the
                # accumulation group once with a bank-covering zero matmul and
                # accumulate everything else with start=False.
                ctx_ps = psum_ctx.tile([P, M, B], F32)
                nc.tensor.matmul(ctx_ps[:], ones_bf[:], zeros_row[:],
                                 start=True, stop=False)
                for t in range(CHUNK_TILES):
                    ptp = psum_tr.tile([P, B], F32, tag="tr")
                    nc.tensor.transpose(ptp[:], p_sb[:, t * P:(t + 1) * P], ident[0:B, 0:B])
                    pts = pt_pool.tile([P, B], BF16)
                    nc.scalar.copy(pts[:], ptp[:])
                    for bb in range(B):
                        for mm in range(M):
                            last = (t == CHUNK_TILES - 1 and bb == B - 1
                                    and mm == M - 1)
                            nc.tensor.matmul(
                                ctx_ps[:, mm, bb:bb + 1],
                                enc_tiles[t][:, bb, mm * P:(mm + 1) * P],
                                pts[:, bb:bb + 1],
                                start=False, stop=last)
                nc.scalar.copy(ctxt_all[:, c, :, :], ctx_ps[:])

            # ---- combine chunks ----
            g_max = persist_pool.tile([B, 1], F32)
            g_negmax = persist_pool.tile([B, 1], F32)
            g_l = persist_pool.tile([B, 1], F32)
            g_rl = persist_pool.tile([B, 1], F32)
            nc.vector.reduce_max(g_max[:], m_all[:], axis=AX.X)
            nc.vector.tensor_scalar_mul(g_negmax[:], g_max[:], -1.0)
            nc.scalar.activation(w_all[:], m_all[:], ACTF.Exp,
                                 bias=g_negmax[:], scale=1.0)
            junk2 = persist_pool.tile([B, nchunk], F32)
            nc.vector.scalar_tensor_tensor(
                out=junk2[:], in0=l_all[:], scalar=1.0, in1=w_all[:],
                op0=ALU.mult, op1=ALU.mult, accum_out=g_l[:])
            nc.vector.reciprocal(g_rl[:], g_l[:])

            # normalized chunk weights: wn[b, c] = w[b, c] / l_total[b]
            w_norm = persist_pool.tile([B, nchunk], F32)
            nc.vector.tensor_scalar(out=w_norm[:], in0=w_all[:],
                                    scalar1=g_rl[:, 0:1], scalar2=None, op0=ALU.mult)
            # broadcast wn along partitions: [128, chunk, b] via selector
            # matmuls (row bb of wn to all partitions, one matmul per b).
            wb = persist_pool.tile([P, nchunk, B], F32)
            for bb in range(B):
                wbp = psum_tr.tile([P, nchunk], F32, tag="tr")
                nc.tensor.matmul(wbp[:], sel[:, bb, :], w_norm[:],
                                 start=True, stop=True)
                nc.scalar.copy(wb[:, :, bb], wbp[:])

            # weighted sum over chunks (still transposed): [128, m, b]
            ctxf = persist_pool.tile([P, M, B], F32)
            for mm in range(M):
                tmp = persist_pool.tile([P, nchunk, B], F32)
                nc.vector.tensor_tensor(out=tmp[:], in0=ctxt_all[:, :, mm, :],
                                        in1=wb[:], op=ALU.mult)
                nc.vector.reduce_sum(
                    ctxf[:, mm, :],
                    tmp[:].rearrange("p c b -> p b c"),
                    axis=AX.X)

            # transpose back to [b, de] and store
            out_sb = persist_pool.tile([B, DE], F32)
            for mm in range(M):
                op_ps = psum_tr.tile([B, P], F32, tag="tr")
                nc.tensor.transpose(op_ps[:], ctxf[:, mm, :], ident[:])
                nc.scalar.copy(out_sb[:, mm * P:(mm + 1) * P], op_ps[:])
            nc.sync.dma_start(out_d[:], out_sb[:])
            _ctx_cm.__exit__(None, None, None)
            _sc_cm.__exit__(None, None, None)
            _tr_cm.__exit__(None, None, None)

    nc.compile()
    if not nc.is_finalized():
        nc.finalize()
    return nc


_NC = None


def kernel(decoder_hidden, encoder_output, W, b):
    global _NC
    if _NC is None:
        _NC = build_nc()
    decoder_hidden = np.ascontiguousarray(decoder_hidden, dtype=np.float32)
    encoder_output = np.ascontiguousarray(encoder_output, dtype=np.float32)
    W = np.ascontiguousarray(W, dtype=np.float32)
    b = np.ascontiguousarray(b, dtype=np.float32)

    in_maps = []
    for i in range(NCORES):
        sl = slice(i * B, (i + 1) * B)
        in_maps.append({
            "decoder_hidden": decoder_hidden[sl],
            "encoder_output": np.ascontiguousarray(encoder_output[:, sl, :]),
            "W": W,
            "b": b,
        })
    res = run_bass_kernel_spmd(_NC, in_maps, core_ids=list(range(NCORES)))
    return np.concatenate([res.results[i]["out"] for i in range(NCORES)], axis=0)

